# revision 1
# baseline (speedup 1.0000x reference)
"""Block-sparse attention on 8 Trainium2 NeuronCores (Bass/Tile SPMD kernel).

Sharding: batch*head_groups across the 8 cores. Core c handles batch c//4 and
heads [4*(c%4), 4*(c%4)+4). Projection weights are sliced per core host-side
(pre-transposed + bf16-cast, packed so each weight is ONE dma); the [16,16]
block mask specializes the compiled program (only kept blocks are computed).
Each core emits a partial output (its 256-wide d-slice pushed through Wo) in
bf16; the host sums the 4 partials per batch in f32 and adds the bias.

Design (driven by the TimelineSim cost model, where matmul cost = N_out only):
  - scores [k, q] via stationary k^T block (K=64), moving q^T runs (N=128/blk)
  - exp on Act engine into bf16 `at` tiles (the overall pacer: ~0.83ns/elem)
  - AV *natural*: stationary = at block [128k,128q], moving = [v_h|1] [128,65]
    -> av psum [128 q, 65] accumulated over kept j (N=65/blk, den for free)
  - batched normalize: 1/den per-region (DVE reciprocal on strided AP), one
    broadcast (0-stride) tensor_tensor multiply per av bank -> norm_sb bf16
  - PE transpose (bf16 identity) norm_sb [128q,64d] -> tp psum bf16 [64,128],
    packed 2 heads x 4 blocks per [128,512]; one DVE 2x copy -> outT
  - final: outT stationary, wo moving, f32 psum -> bf16 sb -> 1 dma per m
PSUM: sc pool 2x[128,1024]f32 (4 banks, also hosts tp tiles), av 2 banks
(65-wide regions packed 7/bank per 4-i-row chunk), work pool 2 banks
(v/qk-p1 projections + final) = 8. The tile list-scheduler overlaps
projection/final PE work under the Act-bound attention automatically.
"""

import time
from contextlib import ExitStack

import ml_dtypes
import numpy as np

import concourse.bass as bass
import concourse.tile as tile
from concourse import bacc, mybir
from concourse.bass_utils import run_bass_kernel_spmd

BF16 = mybir.dt.bfloat16
F32 = mybir.dt.float32
bf16 = ml_dtypes.bfloat16

B, S, D, H = 2, 2048, 1024, 16
DH = 64
BLK = 128
NB = 16
NCORES = 8
HPC = H // (NCORES // B)   # 4 heads per core
E = HPC * DH               # 256 projection columns per core
EV = HPC * (DH + 1)        # 260: v stored as [v_h | 1] per head
KD = D // 128              # 8 contraction chunks
QC = 4                     # query-block rows per attention chunk
NQC = NB // QC             # 4 chunks
FILL = 1024                # score fill width (2 psum banks, 8 block-cols)

_nc_cache: dict = {}
last_run_info: dict = {}
AT_BUFS = 28
TP_IN_WORK = True
V_PRIO = 0
TP_AT_END = False

# NOTE: emission order IS program order for the tile dep tracker — every
# tensor's writer must be emitted before its readers (v before all attention,
# qk1 before a1*, Fg after both g-chunks). high_priority on attention makes
# the low-priority proj/final work execute lazily as PE gap filler anyway.
ORDER = ("qk0", "v", "a00", "a01", "a02", "a03", "qk1", "a10",
         "F0", "a11", "F1", "a12", "F2", "a13", "F3t")


def _order():
    return ORDER


def _runs_of(lst):
    out = []
    for i in lst:
        if out and i == out[-1][-1] + 1:
            out[-1].append(i)
        else:
            out.append([i])
    return out


def _emit(tc, aps, kept):
    nc = tc.nc
    xp_ap, wq_ap, wk_ap, wv_ap, wo_ap, id_ap, outp_ap = aps
    Exp = mybir.ActivationFunctionType.Exp
    first_j = {i: kept[i][0] for i in range(NB)}
    last_j = {i: kept[i][-1] for i in range(NB)}
    col_kept = [[i for i in range(NB) if j in kept[i]] for j in range(NB)]

    with ExitStack() as ctx:
        persist = ctx.enter_context(tc.tile_pool(name="persist", bufs=1))
        sc_ps = ctx.enter_context(tc.tile_pool(name="sc_ps", bufs=2, space="PSUM"))
        work_ps = ctx.enter_context(tc.tile_pool(name="work_ps", bufs=2, space="PSUM"))
        av_ps = ctx.enter_context(tc.tile_pool(name="av_ps", bufs=1, space="PSUM"))
        at_sb = ctx.enter_context(tc.tile_pool(name="at_sb", bufs=AT_BUFS))
        nrm_sb = ctx.enter_context(tc.tile_pool(name="nrm_sb", bufs=4))
        fin_sb = ctx.enter_context(tc.tile_pool(name="fin_sb", bufs=4))

        # ---- input loads: wq then x0 first so q-proj sc0 starts earliest ----
        ident = persist.tile([128, 128], BF16, name="ident", tag="ident")
        wq = persist.tile([128, 2048], BF16, name="wq", tag="wq")
        wk = persist.tile([128, 2048], BF16, name="wk", tag="wk")
        wv = persist.tile([128, 2048], BF16, name="wv", tag="wv")
        wo = persist.tile([128, 2048], BF16, name="wo", tag="wo")
        # x packed [part, sc, kd, s]: one tile per s-group so deps are exact
        xg = [persist.tile([128, 4096], BF16, name=f"xg{g}", tag=f"xg{g}")
              for g in range(4)]
        nc.sync.dma_start(wq[:], wq_ap[:, :])
        nc.sync.dma_start(xg[0][:, 0:2048], xp_ap[:, 0:2048])
        nc.sync.dma_start(xg[0][:, 2048:4096], xp_ap[:, 2048:4096])
        nc.sync.dma_start(wk[:], wk_ap[:, :])
        nc.sync.dma_start(xg[1][:], xp_ap[:, 4096:8192])
        nc.sync.dma_start(wv[:], wv_ap[:, :])
        nc.sync.dma_start(xg[2][:], xp_ap[:, 8192:12288])
        nc.sync.dma_start(wo[:], wo_ap[:, :])
        nc.sync.dma_start(ident[:], id_ap[:, :])
        nc.sync.dma_start(xg[3][:], xp_ap[:, 12288:16384])

        qT = [persist.tile([128, S], BF16, name=f"qT{p}", tag=f"qT{p}") for p in range(2)]
        kT = [persist.tile([128, S], BF16, name=f"kT{p}", tag=f"kT{p}") for p in range(2)]
        vv = [persist.tile([128, EV], BF16, name=f"v{m}", tag=f"v{m}") for m in range(NB)]
        outTbf = [persist.tile([128, S], BF16, name=f"oT{p}", tag=f"oT{p}") for p in range(2)]

        def x_mv(kd, sc):
            # moving x^T slice for contraction chunk kd, s-columns [512*sc, 512*(sc+1))
            return xg[sc][:, kd * 512:(kd + 1) * 512]

        def proj_qk(p):
            # sc outer so each x-group is fully consumed as it arrives off DMA
            for sc in range(4):
                for w, dst in ((wq, qT[p]), (wk, kT[p])):
                    ps = work_ps.tile([128, 512], F32, name="w_ps", tag="w")
                    for kd in range(KD):
                        nc.tensor.matmul(
                            ps[:],
                            w[:, kd * 256 + p * 128: kd * 256 + (p + 1) * 128],
                            x_mv(kd, sc),
                            start=(kd == 0), stop=(kd == KD - 1),
                        )
                    nc.vector.tensor_copy(dst[:, sc * 512:(sc + 1) * 512], ps[:])

        def proj_v(ms=range(NB)):
            from contextlib import nullcontext
            pctx = tc.high_priority(offset=V_PRIO) if V_PRIO else nullcontext()
            with pctx:
                _proj_v_body(ms)

        def _proj_v_body(ms):
            for m in ms:
                ps = work_ps.tile([128, 260], F32, name="v_ps", tag="w")
                for kd in range(KD):
                    nc.tensor.matmul(
                        ps[:, 0:E],
                        xg[m // 4][:, kd * 512 + (m % 4) * 128:
                                   kd * 512 + (m % 4) * 128 + 128],
                        wv[:, kd * 256:(kd + 1) * 256],
                        start=(kd == 0), stop=(kd == KD - 1),
                    )
                v3 = vv[m].rearrange("p (g c) -> p g c", g=HPC)
                nc.any.memset(v3[:, :, 64:65], 1.0)
                nc.vector.tensor_copy(
                    v3[:, :, 0:64],
                    ps[:, 0:E].rearrange("p (g c) -> p g c", g=HPC),
                )

        def attn_chunk(p, qc):
            # attention always outranks proj/final filler in the scheduler's
            # ready heap (dependencies still force projections to run first
            # where needed); emission order still controls psum slot FIFO.
            with tc.high_priority(offset=1_000_000):
                i0 = qc * QC
                irange = list(range(i0, i0 + QC))
                # av accumulators: one bank per head, 4 regions of width 65
                av = [av_ps.tile([128, 512], F32, name=f"av{a}", tag=f"av{a}")
                      for a in range(2)]

                # start=True pending-zeroes the whole 2KB bank (zero region),
                # so only the FIRST matmul touching each bank may use it; all
                # other regions' first writes then read-as-zero via the mark.
                primed = [False, False]

                def av_region(a, li):
                    return a, av[a][:, 65 * li:65 * li + 65]

                # plan flushes: exact 8-block packing (j-groups may split) so
                # exp count is minimal; then emit head A/B flushes interleaved
                # so head A's normalize overlaps head B's remaining flushes
                blocks = [(j, i) for j in range(NB)
                          for i in col_kept[j] if i in irange]
                CAP = FILL // 128
                plans = [blocks[c:c + CAP] for c in range(0, len(blocks), CAP)]

                def emit_qk(a, fl):
                    rows = slice(a * 64, (a + 1) * 64)
                    sc_t = sc_ps.tile([128, FILL], F32, name="sc", tag="sc")
                    # fuse consecutive same-j runs of consecutive i into one
                    # matmul, split at 512-col psum bank boundaries
                    col = 0
                    for j, ks in fl:
                        for run in _runs_of(ks):
                            width = len(run) * 128
                            qcol = run[0] * 128
                            done = 0
                            while done < width:
                                seg = min(width - done, 512 - ((col + done) % 512))
                                nc.tensor.matmul(
                                    sc_t[:, col + done: col + done + seg],
                                    kT[p][rows, j * 128:(j + 1) * 128],
                                    qT[p][rows, qcol + done: qcol + done + seg],
                                )
                                done += seg
                            col += width
                    return sc_t, col

                def emit_exp_av(a, fl, sc_t, colw):
                    h = 2 * p + a
                    at = at_sb.tile([128, FILL], BF16, name="at", tag="at")
                    nc.scalar.activation(at[:, 0:colw], sc_t[:, 0:colw], Exp)
                    off = 0
                    for j, ks in fl:
                        lhs_v = vv[j][:, 65 * h:65 * h + 65]
                        for i in ks:
                            bank_id, region = av_region(a, i - i0)
                            nc.tensor.matmul(
                                region,
                                at[:, off:off + 128],
                                lhs_v,
                                start=not primed[bank_id],
                                stop=(j == last_j[i]),
                                skip_group_check=True,
                            )
                            primed[bank_id] = True
                            off += 128

                for fi, plan in enumerate(plans):
                    # regroup plan entries [(j,i)...] -> [(j, ks)...]
                    fl = []
                    for j, i in plan:
                        if fl and fl[-1][0] == j:
                            fl[-1][1].append(i)
                        else:
                            fl.append((j, [i]))
                    for a in range(2):
                        sc_t, colw = emit_qk(a, fl)
                        emit_exp_av(a, fl, sc_t, colw)

                # ---- normalize + transpose this chunk (per head so head A's
                # transposes unlock while head B's normalize runs) ----
                rc = nrm_sb.tile([128, 8], F32, name="rc", tag="rc")
                nrm = nrm_sb.tile([128, 512], BF16, name="nrm", tag="nrm")
                tp = work_ps.tile([128, 512], BF16, name="tp", tag="w")
                for a in range(2):
                    nc.vector.reciprocal(rc[:, 4 * a:4 * a + 4], av[a][:, 64:260:65])
                    nc.vector.tensor_tensor(
                        nrm[:, a * 256:a * 256 + 256].rearrange("p (r c) -> p r c", c=64),
                        av[a][:, 0:260].rearrange("p (r c) -> p r c", c=65)[:, :, 0:64],
                        rc[:, 4 * a:4 * a + 4].unsqueeze(2).broadcast_to([128, 4, 64]),
                        mybir.AluOpType.mult,
                    )
                    for li in range(QC):
                        idx = a * QC + li
                        nc.tensor.transpose(
                            tp[a * 64:(a + 1) * 64, li * 128:(li + 1) * 128],
                            nrm[:, idx * 64:(idx + 1) * 64],
                            ident[:],
                        )
                nc.vector.tensor_copy(
                    outTbf[p][:, qc * 512:(qc + 1) * 512], tp[:])

        def final(ms, tail=False):
            # mid-attention groups: copies on DVE only (Act must stay free for
            # exp); the post-attention tail group splits copies across engines
            # and borrows the idle av-pool banks for more psum parallelism
            for mi, m in enumerate(ms):
                fsb = fin_sb.tile([128, 1024], BF16, name="fsb", tag="fsb")
                for n in range(2):
                    if tail and (2 * mi + n) % 2 == 1:
                        ps = av_ps.tile([128, 512], F32, name="f_av",
                                        tag=f"av{(2 * mi + n) // 2 % 2}")
                    else:
                        ps = work_ps.tile([128, 512], F32, name="f_ps", tag="w")
                    for p in range(2):
                        nc.tensor.matmul(
                            ps[:],
                            outTbf[p][:, m * 128:(m + 1) * 128],
                            wo[:, p * 1024 + n * 512: p * 1024 + (n + 1) * 512],
                            start=(p == 0), stop=(p == 1),
                        )
                    if tail and n == 1:
                        nc.scalar.copy(fsb[:, n * 512:(n + 1) * 512], ps[:])
                    else:
                        nc.vector.tensor_copy(fsb[:, n * 512:(n + 1) * 512], ps[:])
                nc.sync.dma_start(
                    outp_ap[m * 128:(m + 1) * 128, :], fsb[:])

        # emission order = scheduler priority. Interleave p0/p1 chunks and emit
        # each final m-group right after the (p1,qc) that completes its outT
        # columns, so final matmuls act as PE filler while later chunks stall
        # on the activation engine (exp).
        # Emission order = scheduler priority: attention QK outranks the bulk
        # projections (v, qk p1) so the Act engine is fed scores ASAP; the
        # lower-priority projections + final groups then fill PE stalls.
        # emission order = scheduler priority AND psum slot FIFO order; see
        # ORDER spec tokens: qk0/qk1, vA (m0-7), vB (m8-15), aPQ, F0-F2, F3t
        for tok in _order():
            if tok.startswith("#"):
                continue
            if tok == "qk0":
                proj_qk(0)
            elif tok == "qk1":
                proj_qk(1)
            elif tok == "v":
                proj_v()
            elif tok == "vA":
                proj_v(range(0, 8))
            elif tok == "vB":
                proj_v(range(8, 16))
            elif tok.startswith("a"):
                attn_chunk(int(tok[1]), int(tok[2]))
            elif tok.startswith("F"):
                g = int(tok[1])
                final([4 * g + k for k in range(4)], tail=tok.endswith("t"))
            else:
                raise ValueError(tok)


def _get_nc(kept):
    key = (kept, ORDER)
    if key in _nc_cache:
        return _nc_cache[key]
    nc = bacc.Bacc("TRN2", target_bir_lowering=False, debug=False, num_devices=NCORES)
    xp_ap = nc.dram_tensor("xp", [128, 16384], BF16, kind="ExternalInput").ap()
    wq_ap = nc.dram_tensor("wqp", [128, 2048], BF16, kind="ExternalInput").ap()
    wk_ap = nc.dram_tensor("wkp", [128, 2048], BF16, kind="ExternalInput").ap()
    wv_ap = nc.dram_tensor("wvp", [128, 2048], BF16, kind="ExternalInput").ap()
    wo_ap = nc.dram_tensor("wop", [128, 2048], BF16, kind="ExternalInput").ap()
    id_ap = nc.dram_tensor("ident", [128, 128], BF16, kind="ExternalInput").ap()
    outp_ap = nc.dram_tensor("outp", [S, D], BF16, kind="ExternalOutput").ap()
    with tile.TileContext(nc) as tc:
        _emit(tc, (xp_ap, wq_ap, wk_ap, wv_ap, wo_ap, id_ap, outp_ap), kept)
    nc.compile()
    _nc_cache[key] = nc
    return nc


def _pack_x(xb):
    # x[b].T [1024,2048] -> [part, sc, kd, s-within] -> [128, 16384]
    t = np.ascontiguousarray(xb.T).astype(bf16)          # [1024, 2048]
    t = t.reshape(KD, 128, 4, 512).transpose(1, 2, 0, 3)  # [128, 4, 8, 512]
    return np.ascontiguousarray(t.reshape(128, 16384))


def _pack_w(wslT):
    # W[sl,:].T [1024, 256] -> [128, kd*256]
    t = wslT.reshape(KD, 128, 256).transpose(1, 0, 2)
    return np.ascontiguousarray(t.reshape(128, 2048)).astype(bf16)


def _pack_wo(woT):
    # Wo[:,sl].T [256, 1024] -> [128, p*1024 + outcol]
    t = woT.reshape(2, 128, 1024).transpose(1, 0, 2)
    return np.ascontiguousarray(t.reshape(128, 2048)).astype(bf16)


def kernel(x, Wq, Wk, Wv, Wo, bo, block_mask):
    x = np.asarray(x, dtype=np.float32)
    Wq = np.asarray(Wq, dtype=np.float32)
    Wk = np.asarray(Wk, dtype=np.float32)
    Wv = np.asarray(Wv, dtype=np.float32)
    Wo = np.asarray(Wo, dtype=np.float32)
    bo = np.asarray(bo, dtype=np.float32)
    mask = np.asarray(block_mask).astype(bool)

    kept = tuple(tuple(int(j) for j in np.nonzero(mask[i])[0]) for i in range(NB))
    assert all(len(js) > 0 for js in kept), "a query block row has no kept blocks"

    t0 = time.monotonic()
    nc = _get_nc(kept)
    t_compile = time.monotonic() - t0

    ident = np.eye(128).astype(bf16)
    xp_b = [_pack_x(x[b]) for b in range(B)]
    in_maps = []
    for c in range(NCORES):
        b = c // (NCORES // B)
        hs = c % (NCORES // B)
        sl = slice(hs * E, (hs + 1) * E)
        in_maps.append({
            "xp": xp_b[b],
            "wqp": _pack_w(np.ascontiguousarray(
                (Wq[sl, :] / np.sqrt(np.float32(DH))).T).astype(np.float32)),
            "wkp": _pack_w(np.ascontiguousarray(Wk[sl, :].T).astype(np.float32)),
            "wvp": _pack_w(np.ascontiguousarray(Wv[sl, :].T).astype(np.float32)),
            "wop": _pack_wo(np.ascontiguousarray(Wo[:, sl].T).astype(np.float32)),
            "ident": ident,
        })

    t0 = time.monotonic()
    res = run_bass_kernel_spmd(nc, in_maps, list(range(NCORES)))
    t_run = time.monotonic() - t0

    out = np.zeros((B, S, D), np.float32)
    for c in range(NCORES):
        out[c // (NCORES // B)] += res.results[c]["outp"].astype(np.float32)
    out += bo[None, None, :]

    last_run_info.update(compile_s=t_compile, run_s=t_run, nc=nc)
    return out



# revision 7
# speedup vs baseline: 1.0193x; 1.0193x over previous
"""Block-sparse attention on 8 Trainium2 NeuronCores (Bass/Tile SPMD kernel).

Sharding: batch*head_groups across the 8 cores. Core c handles batch c//4 and
heads [4*(c%4), 4*(c%4)+4). Projection weights are sliced per core host-side
(pre-transposed + bf16-cast, packed so each weight is ONE dma); the [16,16]
block mask specializes the compiled program (only kept blocks are computed).
Each core emits a partial output (its 256-wide d-slice pushed through Wo) in
bf16; the host sums the 4 partials per batch in f32 and adds the bias.

Design (driven by the TimelineSim cost model, where matmul cost = N_out only):
  - scores [k, q] via stationary k^T block (K=64), moving q^T runs (N=128/blk)
  - exp on Act engine into bf16 `at` tiles (the overall pacer: ~0.83ns/elem)
  - AV *natural*: stationary = at block [128k,128q], moving = [v_h|1] [128,65]
    -> av psum [128 q, 65] accumulated over kept j (N=65/blk, den for free)
  - batched normalize: 1/den per-region (DVE reciprocal on strided AP), one
    broadcast (0-stride) tensor_tensor multiply per av bank -> norm_sb bf16
  - PE transpose (bf16 identity) norm_sb [128q,64d] -> tp psum bf16 [64,128],
    packed 2 heads x 4 blocks per [128,512]; one DVE 2x copy -> outT
  - final: outT stationary, wo moving, f32 psum -> bf16 sb -> 1 dma per m
PSUM: sc pool 2x[128,1024]f32 (4 banks, also hosts tp tiles), av 2 banks
(65-wide regions packed 7/bank per 4-i-row chunk), work pool 2 banks
(v/qk-p1 projections + final) = 8. The tile list-scheduler overlaps
projection/final PE work under the Act-bound attention automatically.
"""

import time
from contextlib import ExitStack

import ml_dtypes
import numpy as np

import concourse.bass as bass
import concourse.tile as tile
from concourse import bacc, mybir
from concourse.bass_utils import run_bass_kernel_spmd

BF16 = mybir.dt.bfloat16
F8 = mybir.dt.float8e4
F32 = mybir.dt.float32
bf16 = ml_dtypes.bfloat16
f8 = ml_dtypes.float8_e4m3
DR = mybir.MatmulPerfMode.DoubleRow
WS = 32.0               # fp8 weight upscale; exp scale compensates (2^-13)

B, S, D, H = 2, 2048, 1024, 16
DH = 64
BLK = 128
NB = 16
NCORES = 8
HPC = H // (NCORES // B)   # 4 heads per core
E = HPC * DH               # 256 projection columns per core
EV = HPC * (DH + 1)        # 260: v stored as [v_h | 1] per head
KD = D // 128              # 8 contraction chunks
QC = 4                     # query-block rows per attention chunk
NQC = NB // QC             # 4 chunks
FILL = 1024                # score fill width (2 psum banks, 8 block-cols)

_nc_cache: dict = {}
last_run_info: dict = {}
AT_BUFS = 28
TP_IN_WORK = True
V_PRIO = 0
TP_AT_END = False

# NOTE: emission order IS program order for the tile dep tracker — every
# tensor's writer must be emitted before its readers (v before all attention,
# qk1 before a1*, Fg after both g-chunks). high_priority on attention makes
# the low-priority proj/final work execute lazily as PE gap filler anyway.
ORDER = ("qk0", "v", "a00", "a01", "a02", "a03", "qk1", "a10",
         "F0", "a11", "F1", "a12", "F2", "a13", "F3t")


def _order():
    return ORDER


def _runs_of(lst):
    out = []
    for i in lst:
        if out and i == out[-1][-1] + 1:
            out[-1].append(i)
        else:
            out.append([i])
    return out


def _emit(tc, aps, kept):
    nc = tc.nc
    xhi_ap, xlo_ap, wq_ap, wk_ap, wv_ap, wo_ap, id_ap, outp_ap = aps
    Exp = mybir.ActivationFunctionType.Exp
    first_j = {i: kept[i][0] for i in range(NB)}
    last_j = {i: kept[i][-1] for i in range(NB)}
    col_kept = [[i for i in range(NB) if j in kept[i]] for j in range(NB)]

    with ExitStack() as ctx:
        persist = ctx.enter_context(tc.tile_pool(name="persist", bufs=1))
        sc_ps = ctx.enter_context(tc.tile_pool(name="sc_ps", bufs=2, space="PSUM"))
        work_ps = ctx.enter_context(tc.tile_pool(name="work_ps", bufs=2, space="PSUM"))
        av_ps = ctx.enter_context(tc.tile_pool(name="av_ps", bufs=1, space="PSUM"))
        at_sb = ctx.enter_context(tc.tile_pool(name="at_sb", bufs=AT_BUFS))
        nrm_sb = ctx.enter_context(tc.tile_pool(name="nrm_sb", bufs=4))
        fin_sb = ctx.enter_context(tc.tile_pool(name="fin_sb", bufs=4))

        # ---- input loads: wq then xhi0 first so q-proj sc0 starts earliest.
        # Weight packs are [whi | wlo/16 | whi/16] fp8, each 2048 wide; x comes
        # as fp8 hi + (residual*16) lo so projections run 3-term compensated
        # DoubleRow fp8 at ~bf16 accuracy (dropped term is xlo*wlo/256).
        ident = persist.tile([128, 128], BF16, name="ident", tag="ident")
        wq = persist.tile([128, 6144], F8, name="wq", tag="wq")
        wk = persist.tile([128, 6144], F8, name="wk", tag="wk")
        wv = persist.tile([128, 6144], F8, name="wv", tag="wv")
        wo = persist.tile([128, 2048], BF16, name="wo", tag="wo")
        # x packed [part, sc, kd, s]: one tile per s-group so deps are exact
        xg = [persist.tile([128, 4096], F8, name=f"xg{g}", tag=f"xg{g}")
              for g in range(4)]
        xl = [persist.tile([128, 4096], F8, name=f"xl{g}", tag=f"xl{g}")
              for g in range(4)]
        nc.sync.dma_start(wq[:], wq_ap[:, :])
        nc.sync.dma_start(xg[0][:], xhi_ap[:, 0:4096])
        nc.sync.dma_start(wk[:], wk_ap[:, :])
        nc.sync.dma_start(xg[1][:], xhi_ap[:, 4096:8192])
        nc.sync.dma_start(wv[:], wv_ap[:, :])
        nc.sync.dma_start(xg[2][:], xhi_ap[:, 8192:12288])
        nc.sync.dma_start(xg[3][:], xhi_ap[:, 12288:16384])
        nc.sync.dma_start(xl[0][:], xlo_ap[:, 0:4096])
        nc.sync.dma_start(xl[1][:], xlo_ap[:, 4096:8192])
        nc.sync.dma_start(wo[:], wo_ap[:, :])
        nc.sync.dma_start(ident[:], id_ap[:, :])
        nc.sync.dma_start(xl[2][:], xlo_ap[:, 8192:12288])
        nc.sync.dma_start(xl[3][:], xlo_ap[:, 12288:16384])

        qT = [persist.tile([128, S], BF16, name=f"qT{p}", tag=f"qT{p}") for p in range(2)]
        kT = [persist.tile([128, S], BF16, name=f"kT{p}", tag=f"kT{p}") for p in range(2)]
        vv = [persist.tile([128, EV], BF16, name=f"v{m}", tag=f"v{m}") for m in range(NB)]
        outTbf = [persist.tile([128, S], BF16, name=f"oT{p}", tag=f"oT{p}") for p in range(2)]

        # 3D views: w packs [part, kd(8), 256]; x groups [part, kd(8), 512]
        wq3 = [wq[:, t * 2048:(t + 1) * 2048].rearrange("p (kd c) -> p kd c", c=256)
               for t in range(3)]
        wk3 = [wk[:, t * 2048:(t + 1) * 2048].rearrange("p (kd c) -> p kd c", c=256)
               for t in range(3)]
        wv3 = [wv[:, t * 2048:(t + 1) * 2048].rearrange("p (kd c) -> p kd c", c=256)
               for t in range(3)]
        xg3 = [g.rearrange("p (kd s) -> p kd s", s=512) for g in xg]
        xl3 = [g.rearrange("p (kd s) -> p kd s", s=512) for g in xl]

        def proj_qk(p):
            # sc outer so each x-group is fully consumed as it arrives off DMA
            # 3-term compensated fp8 DoubleRow: terms (xhi*whi, xhi*wlo16,
            # xlo*whi16); kd pairs via DoubleRow; N capped at 256 (2N<=512).
            for sc in range(4):
                for w3, dst in ((wq3, qT[p]), (wk3, kT[p])):
                    ps = work_ps.tile([128, 512], F32, name="w_ps", tag="w")
                    nmm = 3 * 4 * 2
                    i = 0
                    for t, xs in ((0, xg3), (1, xg3), (2, xl3)):
                        for kdp in range(4):
                            lhs = w3[t][:, 2 * kdp:2 * kdp + 2,
                                        p * 128:(p + 1) * 128]
                            for hh in range(2):
                                nc.tensor.matmul(
                                    ps[:, hh * 256:(hh + 1) * 256],
                                    lhs,
                                    xs[sc][:, 2 * kdp:2 * kdp + 2,
                                           hh * 256:(hh + 1) * 256],
                                    start=(i == 0), stop=(i == nmm - 1),
                                    perf_mode=DR,
                                )
                                i += 1
                    nc.vector.tensor_copy(dst[:, sc * 512:(sc + 1) * 512], ps[:])

        def proj_v(ms=range(NB)):
            from contextlib import nullcontext
            pctx = tc.high_priority(offset=V_PRIO) if V_PRIO else nullcontext()
            with pctx:
                _proj_v_body(ms)

        def _proj_v_body(ms):
            for m in ms:
                ps = work_ps.tile([128, 260], F32, name="v_ps", tag="w")
                nmm = 3 * 4
                i = 0
                for t, xs in ((0, xg3), (1, xg3), (2, xl3)):
                    for kdp in range(4):
                        nc.tensor.matmul(
                            ps[:, 0:E],
                            xs[m // 4][:, 2 * kdp:2 * kdp + 2,
                                       (m % 4) * 128:(m % 4) * 128 + 128],
                            wv3[t][:, 2 * kdp:2 * kdp + 2, 0:256],
                            start=(i == 0), stop=(i == nmm - 1),
                            perf_mode=DR,
                        )
                        i += 1
                v3 = vv[m].rearrange("p (g c) -> p g c", g=HPC)
                nc.any.memset(v3[:, :, 64:65], 1.0)
                nc.vector.tensor_copy(
                    v3[:, :, 0:64],
                    ps[:, 0:E].rearrange("p (g c) -> p g c", g=HPC),
                )

        def attn_chunk(p, qc):
            # attention always outranks proj/final filler in the scheduler's
            # ready heap (dependencies still force projections to run first
            # where needed); emission order still controls psum slot FIFO.
            with tc.high_priority(offset=1_000_000):
                i0 = qc * QC
                irange = list(range(i0, i0 + QC))
                # av accumulators: one bank per head, 4 regions of width 65
                av = [av_ps.tile([128, 512], F32, name=f"av{a}", tag=f"av{a}")
                      for a in range(2)]

                # start=True pending-zeroes the whole 2KB bank (zero region),
                # so only the FIRST matmul touching each bank may use it; all
                # other regions' first writes then read-as-zero via the mark.
                primed = [False, False]

                def av_region(a, li):
                    return a, av[a][:, 65 * li:65 * li + 65]

                # plan flushes: exact 8-block packing (j-groups may split) so
                # exp count is minimal; then emit head A/B flushes interleaved
                # so head A's normalize overlaps head B's remaining flushes
                blocks = [(j, i) for j in range(NB)
                          for i in col_kept[j] if i in irange]
                CAP = FILL // 128
                plans = [blocks[c:c + CAP] for c in range(0, len(blocks), CAP)]

                def emit_qk(a, fl):
                    rows = slice(a * 64, (a + 1) * 64)
                    sc_t = sc_ps.tile([128, FILL], F32, name="sc", tag="sc")
                    # fuse consecutive same-j runs of consecutive i into one
                    # matmul, split at 512-col psum bank boundaries
                    col = 0
                    for j, ks in fl:
                        for run in _runs_of(ks):
                            width = len(run) * 128
                            qcol = run[0] * 128
                            done = 0
                            while done < width:
                                seg = min(width - done, 512 - ((col + done) % 512))
                                nc.tensor.matmul(
                                    sc_t[:, col + done: col + done + seg],
                                    kT[p][rows, j * 128:(j + 1) * 128],
                                    qT[p][rows, qcol + done: qcol + done + seg],
                                )
                                done += seg
                            col += width
                    return sc_t, col

                def emit_exp_av(a, fl, sc_t, colw):
                    h = 2 * p + a
                    at = at_sb.tile([128, FILL], BF16, name="at", tag="at")
                    # scale = 1/(WS*WS*sqrt(DH)): undo fp8 weight upscale and
                    # apply the attention 1/sqrt(dh) before exp
                    nc.scalar.activation(at[:, 0:colw], sc_t[:, 0:colw], Exp,
                                         scale=2.0 ** -13)
                    off = 0
                    for j, ks in fl:
                        lhs_v = vv[j][:, 65 * h:65 * h + 65]
                        for i in ks:
                            bank_id, region = av_region(a, i - i0)
                            nc.tensor.matmul(
                                region,
                                at[:, off:off + 128],
                                lhs_v,
                                start=not primed[bank_id],
                                stop=(j == last_j[i]),
                                skip_group_check=True,
                            )
                            primed[bank_id] = True
                            off += 128

                for fi, plan in enumerate(plans):
                    # regroup plan entries [(j,i)...] -> [(j, ks)...]
                    fl = []
                    for j, i in plan:
                        if fl and fl[-1][0] == j:
                            fl[-1][1].append(i)
                        else:
                            fl.append((j, [i]))
                    for a in range(2):
                        sc_t, colw = emit_qk(a, fl)
                        emit_exp_av(a, fl, sc_t, colw)

                # ---- normalize + transpose this chunk (per head so head A's
                # transposes unlock while head B's normalize runs) ----
                rc = nrm_sb.tile([128, 8], F32, name="rc", tag="rc")
                nrm = nrm_sb.tile([128, 512], BF16, name="nrm", tag="nrm")
                tp = work_ps.tile([128, 512], BF16, name="tp", tag="w")
                for a in range(2):
                    nc.vector.reciprocal(rc[:, 4 * a:4 * a + 4], av[a][:, 64:260:65])
                    nc.vector.tensor_tensor(
                        nrm[:, a * 256:a * 256 + 256].rearrange("p (r c) -> p r c", c=64),
                        av[a][:, 0:260].rearrange("p (r c) -> p r c", c=65)[:, :, 0:64],
                        rc[:, 4 * a:4 * a + 4].unsqueeze(2).broadcast_to([128, 4, 64]),
                        mybir.AluOpType.mult,
                    )
                    for li in range(QC):
                        idx = a * QC + li
                        nc.tensor.transpose(
                            tp[a * 64:(a + 1) * 64, li * 128:(li + 1) * 128],
                            nrm[:, idx * 64:(idx + 1) * 64],
                            ident[:],
                        )
                nc.vector.tensor_copy(
                    outTbf[p][:, qc * 512:(qc + 1) * 512], tp[:])

        def final(ms, tail=False):
            # mid-attention groups: copies on DVE only (Act must stay free for
            # exp); the post-attention tail group splits copies across engines
            # and borrows the idle av-pool banks for more psum parallelism
            for mi, m in enumerate(ms):
                fsb = fin_sb.tile([128, 1024], BF16, name="fsb", tag="fsb")
                for n in range(2):
                    if tail and (2 * mi + n) % 2 == 1:
                        ps = av_ps.tile([128, 512], F32, name="f_av",
                                        tag=f"av{(2 * mi + n) // 2 % 2}")
                    else:
                        ps = work_ps.tile([128, 512], F32, name="f_ps", tag="w")
                    for p in range(2):
                        nc.tensor.matmul(
                            ps[:],
                            outTbf[p][:, m * 128:(m + 1) * 128],
                            wo[:, p * 1024 + n * 512: p * 1024 + (n + 1) * 512],
                            start=(p == 0), stop=(p == 1),
                        )
                    if tail and n == 1:
                        nc.scalar.copy(fsb[:, n * 512:(n + 1) * 512], ps[:])
                    else:
                        nc.vector.tensor_copy(fsb[:, n * 512:(n + 1) * 512], ps[:])
                nc.sync.dma_start(
                    outp_ap[m * 128:(m + 1) * 128, :], fsb[:])

        # emission order = scheduler priority. Interleave p0/p1 chunks and emit
        # each final m-group right after the (p1,qc) that completes its outT
        # columns, so final matmuls act as PE filler while later chunks stall
        # on the activation engine (exp).
        # Emission order = scheduler priority: attention QK outranks the bulk
        # projections (v, qk p1) so the Act engine is fed scores ASAP; the
        # lower-priority projections + final groups then fill PE stalls.
        # emission order = scheduler priority AND psum slot FIFO order; see
        # ORDER spec tokens: qk0/qk1, vA (m0-7), vB (m8-15), aPQ, F0-F2, F3t
        for tok in _order():
            if tok.startswith("#"):
                continue
            if tok == "qk0":
                proj_qk(0)
            elif tok == "qk1":
                proj_qk(1)
            elif tok == "v":
                proj_v()
            elif tok == "vA":
                proj_v(range(0, 8))
            elif tok == "vB":
                proj_v(range(8, 16))
            elif tok.startswith("a"):
                attn_chunk(int(tok[1]), int(tok[2]))
            elif tok.startswith("F"):
                g = int(tok[1])
                final([4 * g + k for k in range(4)], tail=tok.endswith("t"))
            else:
                raise ValueError(tok)


def _get_nc(kept):
    key = (kept, ORDER)
    if key in _nc_cache:
        return _nc_cache[key]
    nc = bacc.Bacc("TRN2", target_bir_lowering=False, debug=False, num_devices=NCORES)
    xhi_ap = nc.dram_tensor("xhi", [128, 16384], F8, kind="ExternalInput").ap()
    xlo_ap = nc.dram_tensor("xlo", [128, 16384], F8, kind="ExternalInput").ap()
    wq_ap = nc.dram_tensor("wqp", [128, 6144], F8, kind="ExternalInput").ap()
    wk_ap = nc.dram_tensor("wkp", [128, 6144], F8, kind="ExternalInput").ap()
    wv_ap = nc.dram_tensor("wvp", [128, 6144], F8, kind="ExternalInput").ap()
    wo_ap = nc.dram_tensor("wop", [128, 2048], BF16, kind="ExternalInput").ap()
    id_ap = nc.dram_tensor("ident", [128, 128], BF16, kind="ExternalInput").ap()
    outp_ap = nc.dram_tensor("outp", [S, D], BF16, kind="ExternalOutput").ap()
    with tile.TileContext(nc) as tc:
        _emit(tc, (xhi_ap, xlo_ap, wq_ap, wk_ap, wv_ap, wo_ap, id_ap, outp_ap),
              kept)
    nc.compile()
    _nc_cache[key] = nc
    return nc


def _pack_x_layout(t):
    # x[b].T f32 [1024,2048] -> [part, sc, kd, s-within] -> [128, 16384]
    t = t.reshape(KD, 128, 4, 512).transpose(1, 2, 0, 3)  # [128, 4, 8, 512]
    return np.ascontiguousarray(t.reshape(128, 16384))


def _pack_x_hilo(xb):
    a = np.ascontiguousarray(xb.T).astype(np.float32)     # [1024, 2048]
    hi = a.astype(f8)
    lo = ((a - hi.astype(np.float32)) * 16.0).astype(f8)
    return (_pack_x_layout(hi.astype(np.float32)).astype(f8),
            _pack_x_layout(lo.astype(np.float32)).astype(f8))


def _pack_w_layout(t):
    # [1024, 256] f32 -> [128, kd*256] f32
    t = t.reshape(KD, 128, 256).transpose(1, 0, 2)
    return np.ascontiguousarray(t.reshape(128, 2048))


def _pack_w_comp(wslT):
    # W[sl,:].T [1024, 256] -> [whi | wlo/16 | whi/16] fp8 [128, 6144]
    a = (wslT * WS).astype(np.float32)
    whi = a.astype(f8)
    wlo16 = ((a - whi.astype(np.float32)) * 16.0).astype(f8)
    p2 = (wlo16.astype(np.float32) / 16.0).astype(f8)
    p3 = (whi.astype(np.float32) / 16.0).astype(f8)
    out = np.concatenate([
        _pack_w_layout(whi.astype(np.float32)),
        _pack_w_layout(p2.astype(np.float32)),
        _pack_w_layout(p3.astype(np.float32)),
    ], axis=1)
    return np.ascontiguousarray(out).astype(f8)


def _pack_wo(woT):
    # Wo[:,sl].T [256, 1024] -> [128, p*1024 + outcol]
    t = woT.reshape(2, 128, 1024).transpose(1, 0, 2)
    return np.ascontiguousarray(t.reshape(128, 2048)).astype(bf16)


def kernel(x, Wq, Wk, Wv, Wo, bo, block_mask):
    x = np.asarray(x, dtype=np.float32)
    Wq = np.asarray(Wq, dtype=np.float32)
    Wk = np.asarray(Wk, dtype=np.float32)
    Wv = np.asarray(Wv, dtype=np.float32)
    Wo = np.asarray(Wo, dtype=np.float32)
    bo = np.asarray(bo, dtype=np.float32)
    mask = np.asarray(block_mask).astype(bool)

    kept = tuple(tuple(int(j) for j in np.nonzero(mask[i])[0]) for i in range(NB))
    assert all(len(js) > 0 for js in kept), "a query block row has no kept blocks"

    t0 = time.monotonic()
    nc = _get_nc(kept)
    t_compile = time.monotonic() - t0

    ident = np.eye(128).astype(bf16)
    xp_b = [_pack_x_hilo(x[b]) for b in range(B)]
    in_maps = []
    for c in range(NCORES):
        b = c // (NCORES // B)
        hs = c % (NCORES // B)
        sl = slice(hs * E, (hs + 1) * E)
        in_maps.append({
            "xhi": xp_b[b][0],
            "xlo": xp_b[b][1],
            "wqp": _pack_w_comp(np.ascontiguousarray(Wq[sl, :].T).astype(np.float32)),
            "wkp": _pack_w_comp(np.ascontiguousarray(Wk[sl, :].T).astype(np.float32)),
            "wvp": _pack_w_comp(np.ascontiguousarray(Wv[sl, :].T).astype(np.float32)),
            "wop": _pack_wo(np.ascontiguousarray(
                (Wo[:, sl] / WS).T).astype(np.float32)),
            "ident": ident,
        })

    t0 = time.monotonic()
    res = run_bass_kernel_spmd(nc, in_maps, list(range(NCORES)))
    t_run = time.monotonic() - t0

    out = np.zeros((B, S, D), np.float32)
    for c in range(NCORES):
        out[c // (NCORES // B)] += res.results[c]["outp"].astype(np.float32)
    out += bo[None, None, :]

    last_run_info.update(compile_s=t_compile, run_s=t_run, nc=nc)
    return out



# revision 26
# speedup vs baseline: 1.1364x; 1.1149x over previous
"""Block-sparse attention on 8 Trainium2 NeuronCores (Bass/Tile SPMD kernel).

Sharding: batch*head_groups across the 8 cores. Core c handles batch c//4 and
heads [4*(c%4), 4*(c%4)+4). Projection weights are sliced per core host-side
(pre-transposed + bf16-cast, packed so each weight is ONE dma); the [16,16]
block mask specializes the compiled program (only kept blocks are computed).
Each core emits a partial output (its 256-wide d-slice pushed through Wo) in
bf16; the host sums the 4 partials per batch in f32 and adds the bias.

Design (driven by the TimelineSim cost model, where matmul cost = N_out only):
  - scores [k, q] via stationary k^T block (K=64), moving q^T runs (N=128/blk)
  - exp on Act engine into bf16 `at` tiles (the overall pacer: ~0.83ns/elem)
  - AV *natural*: stationary = at block [128k,128q], moving = [v_h|1] [128,65]
    -> av psum [128 q, 65] accumulated over kept j (N=65/blk, den for free)
  - batched normalize: 1/den per-region (DVE reciprocal on strided AP), one
    broadcast (0-stride) tensor_tensor multiply per av bank -> norm_sb bf16
  - PE transpose (bf16 identity) norm_sb [128q,64d] -> tp psum bf16 [64,128],
    packed 2 heads x 4 blocks per [128,512]; one DVE 2x copy -> outT
  - final: outT stationary, wo moving, f32 psum -> bf16 sb -> 1 dma per m
PSUM: sc pool 2x[128,1024]f32 (4 banks, also hosts tp tiles), av 2 banks
(65-wide regions packed 7/bank per 4-i-row chunk), work pool 2 banks
(v/qk-p1 projections + final) = 8. The tile list-scheduler overlaps
projection/final PE work under the Act-bound attention automatically.
"""

import time
from contextlib import ExitStack

import ml_dtypes
import numpy as np

import concourse.bass as bass
import concourse.tile as tile
from concourse import bacc, mybir
from concourse.bass_utils import run_bass_kernel_spmd

BF16 = mybir.dt.bfloat16
F8 = mybir.dt.float8e4
F32 = mybir.dt.float32
bf16 = ml_dtypes.bfloat16
f8 = ml_dtypes.float8_e4m3
DR = mybir.MatmulPerfMode.DoubleRow
WS = 32.0               # fp8 weight upscale; exp scale compensates (2^-13)

B, S, D, H = 2, 2048, 1024, 16
DH = 64
BLK = 128
NB = 16
NCORES = 8
HPC = H // (NCORES // B)   # 4 heads per core
E = HPC * DH               # 256 projection columns per core
EV = HPC * (DH + 1)        # 260: v stored as [v_h | 1] per head
KD = D // 128              # 8 contraction chunks
QC = 4                     # query-block rows per attention chunk
NQC = NB // QC             # 4 chunks
FILL = 1024                # score fill width (2 psum banks, 8 block-cols)

_nc_cache: dict = {}
last_run_info: dict = {}
AT_BUFS = 28
TP_IN_WORK = True
V_PRIO = 0
TP_AT_END = False

# NOTE: emission order IS program order for the tile dep tracker — every
# tensor's writer must be emitted before its readers (v before all attention,
# qk1 before a1*, Fg after both g-chunks). high_priority on attention makes
# the low-priority proj/final work execute lazily as PE gap filler anyway.
ORDER = ("qk0", "v", "qk1", "a00", "a01", "a02", "a03", "a10",
         "F0", "a11", "F1", "a12", "F2", "a13", "F3t")


def _order():
    return ORDER


def _runs_of(lst):
    out = []
    for i in lst:
        if out and i == out[-1][-1] + 1:
            out[-1].append(i)
        else:
            out.append([i])
    return out


def _emit(tc, aps, kept):
    nc = tc.nc
    xhi_ap, xlo_ap, wq_ap, wk_ap, wv_ap, wo_ap, id_ap, outp_ap = aps
    Exp = mybir.ActivationFunctionType.Exp
    first_j = {i: kept[i][0] for i in range(NB)}
    last_j = {i: kept[i][-1] for i in range(NB)}
    col_kept = [[i for i in range(NB) if j in kept[i]] for j in range(NB)]

    with ExitStack() as ctx:
        persist = ctx.enter_context(tc.tile_pool(name="persist", bufs=1))
        sc_ps = ctx.enter_context(tc.tile_pool(name="sc_ps", bufs=2, space="PSUM"))
        work_ps = ctx.enter_context(tc.tile_pool(name="work_ps", bufs=2, space="PSUM"))
        av_ps = ctx.enter_context(tc.tile_pool(name="av_ps", bufs=1, space="PSUM"))
        at_sb = ctx.enter_context(tc.tile_pool(name="at_sb", bufs=AT_BUFS))
        nrm_sb = ctx.enter_context(tc.tile_pool(name="nrm_sb", bufs=4))
        fin_sb = ctx.enter_context(tc.tile_pool(name="fin_sb", bufs=4))

        # ---- input loads: wq then xhi0 first so q-proj sc0 starts earliest.
        # Weight packs are [whi | wlo/16 | whi/16] fp8, each 2048 wide; x comes
        # as fp8 hi + (residual*16) lo so projections run 3-term compensated
        # DoubleRow fp8 at ~bf16 accuracy (dropped term is xlo*wlo/256).
        ident = persist.tile([128, 128], BF16, name="ident", tag="ident")
        # q/k weight packs split by p-half so the p0 pack (which gates the
        # first attention flush) is a smaller, earlier DMA
        wq = [persist.tile([128, 3072], F8, name=f"wq{p}", tag=f"wq{p}")
              for p in range(2)]
        wk = [persist.tile([128, 3072], F8, name=f"wk{p}", tag=f"wk{p}")
              for p in range(2)]
        wv = persist.tile([128, 6144], F8, name="wv", tag="wv")
        wo = persist.tile([128, 2048], BF16, name="wo", tag="wo")
        # x packed [part, (hi|lo), sc, kd, s]: one tile per s-group (hi cols
        # 0:4096 + lo 4096:8192) so each group lands in one DMA and its q/k
        # psum groups can close (T3 needs lo) as soon as the group arrives.
        xb = [persist.tile([128, 8192], F8, name=f"xb{g}", tag=f"xb{g}")
              for g in range(4)]
        nc.sync.dma_start(ident[:], id_ap[:, :])
        nc.sync.dma_start(xb[0][:, 0:4096], xhi_ap[:, 0:4096])
        nc.sync.dma_start(wk[0][:], wk_ap[:, 0:3072])
        nc.sync.dma_start(xb[0][:, 4096:8192], xlo_ap[:, 0:4096])
        nc.sync.dma_start(wq[0][:], wq_ap[:, 0:3072])
        nc.sync.dma_start(xb[1][:, 0:4096], xhi_ap[:, 4096:8192])
        nc.sync.dma_start(xb[1][:, 4096:8192], xlo_ap[:, 4096:8192])
        nc.sync.dma_start(xb[2][:, 0:4096], xhi_ap[:, 8192:12288])
        nc.sync.dma_start(xb[2][:, 4096:8192], xlo_ap[:, 8192:12288])
        nc.sync.dma_start(xb[3][:, 0:4096], xhi_ap[:, 12288:16384])
        nc.sync.dma_start(xb[3][:, 4096:8192], xlo_ap[:, 12288:16384])
        nc.sync.dma_start(wv[:], wv_ap[:, :])
        nc.sync.dma_start(wq[1][:], wq_ap[:, 3072:6144])
        nc.sync.dma_start(wk[1][:], wk_ap[:, 3072:6144])
        nc.sync.dma_start(wo[:], wo_ap[:, :])

        # PE p-state warmup: the cost model runs matmuls at 1.5-2x cycle time
        # until the tensor engine has been continuously busy for 3us. Dummy
        # matmuls on ident (first DMA, lands ~1.4us) ramp the clock while the
        # real inputs are still in flight; lowest priority so real work
        # preempts the leftovers.
        wu_ps = av_ps.tile([128, 512], F32, name="wu_ps", tag="av0")
        with tc.high_priority(offset=-1_000_000):
            for _ in range(40):
                nc.tensor.matmul(wu_ps[:, 0:128], ident[:], ident[:],
                                 start=True, stop=True, skip_group_check=True)

        qT = [persist.tile([128, S], BF16, name=f"qT{p}", tag=f"qT{p}") for p in range(2)]
        kT = [persist.tile([128, S], BF16, name=f"kT{p}", tag=f"kT{p}") for p in range(2)]
        vv = [persist.tile([128, EV], BF16, name=f"v{m}", tag=f"v{m}") for m in range(NB)]
        outTbf = [persist.tile([128, S], BF16, name=f"oT{p}", tag=f"oT{p}") for p in range(2)]

        # 3D views: q/k packs per p [part, t, kd(8), 128]; wv [part, t, kd, 256]
        wq3 = [[wq[p][:, t * 1024:(t + 1) * 1024].rearrange(
                "p (kd c) -> p kd c", c=128) for t in range(3)] for p in range(2)]
        wk3 = [[wk[p][:, t * 1024:(t + 1) * 1024].rearrange(
                "p (kd c) -> p kd c", c=128) for t in range(3)] for p in range(2)]
        wv3 = [wv[:, t * 2048:(t + 1) * 2048].rearrange("p (kd c) -> p kd c", c=256)
               for t in range(3)]
        xg3 = [g[:, 0:4096].rearrange("p (kd s) -> p kd s", s=512) for g in xb]
        xl3 = [g[:, 4096:8192].rearrange("p (kd s) -> p kd s", s=512) for g in xb]

        def proj_qk(p):
            # sc outer so each x-group is fully consumed as it arrives off DMA
            # 3-term compensated fp8 DoubleRow: terms (xhi*whi, xhi*wlo16,
            # xlo*whi16); kd pairs via DoubleRow; N capped at 256 (2N<=512).
            for sc in range(4):
                for w3, dst in ((wq3[p], qT[p]), (wk3[p], kT[p])):
                    ps = work_ps.tile([128, 512], F32, name="w_ps", tag="w")
                    nmm = 3 * 4
                    i = 0
                    for t, xs in ((0, xg3), (1, xg3), (2, xl3)):
                        for kdp in range(4):
                            nc.tensor.matmul(
                                ps[:],
                                w3[t][:, 2 * kdp:2 * kdp + 2, :],
                                xs[sc][:, 2 * kdp:2 * kdp + 2, 0:512],
                                start=(i == 0), stop=(i == nmm - 1),
                                perf_mode=DR,
                            )
                            i += 1
                    nc.vector.tensor_copy(dst[:, sc * 512:(sc + 1) * 512], ps[:])

        def proj_v(ms=range(NB)):
            from contextlib import nullcontext
            pctx = tc.high_priority(offset=V_PRIO) if V_PRIO else nullcontext()
            with pctx:
                _proj_v_body(ms)

        def _proj_v_body(ms):
            for m in ms:
                ps = work_ps.tile([128, 260], F32, name="v_ps", tag="w")
                nmm = 3 * 4
                i = 0
                for t, xs in ((0, xg3), (1, xg3), (2, xl3)):
                    for kdp in range(4):
                        nc.tensor.matmul(
                            ps[:, 0:E],
                            xs[m // 4][:, 2 * kdp:2 * kdp + 2,
                                       (m % 4) * 128:(m % 4) * 128 + 128],
                            wv3[t][:, 2 * kdp:2 * kdp + 2, 0:256],
                            start=(i == 0), stop=(i == nmm - 1),
                            perf_mode=DR,
                        )
                        i += 1
                v3 = vv[m].rearrange("p (g c) -> p g c", g=HPC)
                nc.any.memset(v3[:, :, 64:65], 1.0)
                nc.vector.tensor_copy(
                    v3[:, :, 0:64],
                    ps[:, 0:E].rearrange("p (g c) -> p g c", g=HPC),
                )

        def attn_chunk(p, qc):
            # attention always outranks proj/final filler in the scheduler's
            # ready heap (dependencies still force projections to run first
            # where needed); emission order still controls psum slot FIFO.
            with tc.high_priority(offset=1_000_000):
                i0 = qc * QC
                irange = list(range(i0, i0 + QC))
                # av accumulators: one bank per head, 4 regions of width 65
                av = [av_ps.tile([128, 512], F32, name=f"av{a}", tag=f"av{a}")
                      for a in range(2)]

                # start=True pending-zeroes the whole 2KB bank (zero region),
                # so only the FIRST matmul touching each bank may use it; all
                # other regions' first writes then read-as-zero via the mark.
                primed = [False, False]

                def av_region(a, li):
                    return a, av[a][:, 65 * li:65 * li + 65]

                # plan flushes: exact 8-block packing (j-groups may split) so
                # exp count is minimal; then emit head A/B flushes interleaved
                # so head A's normalize overlaps head B's remaining flushes
                blocks = [(j, i) for j in range(NB)
                          for i in col_kept[j] if i in irange]
                CAP = FILL // 128
                plans = [blocks[c:c + CAP] for c in range(0, len(blocks), CAP)]

                def emit_qk(a, fl):
                    rows = slice(a * 64, (a + 1) * 64)
                    sc_t = sc_ps.tile([128, FILL], F32, name="sc", tag="sc")
                    # fuse consecutive same-j runs of consecutive i into one
                    # matmul, split at 512-col psum bank boundaries
                    col = 0
                    for j, ks in fl:
                        for run in _runs_of(ks):
                            width = len(run) * 128
                            qcol = run[0] * 128
                            done = 0
                            while done < width:
                                seg = min(width - done, 512 - ((col + done) % 512))
                                nc.tensor.matmul(
                                    sc_t[:, col + done: col + done + seg],
                                    kT[p][rows, j * 128:(j + 1) * 128],
                                    qT[p][rows, qcol + done: qcol + done + seg],
                                )
                                done += seg
                            col += width
                    return sc_t, col

                def emit_exp_av(a, fl, sc_t, colw):
                    h = 2 * p + a
                    at = at_sb.tile([128, FILL], BF16, name="at", tag="at")
                    # scale = 1/(WS*WS*sqrt(DH)): undo fp8 weight upscale and
                    # apply the attention 1/sqrt(dh) before exp
                    nc.scalar.activation(at[:, 0:colw], sc_t[:, 0:colw], Exp,
                                         scale=2.0 ** -13)
                    off = 0
                    for j, ks in fl:
                        lhs_v = vv[j][:, 65 * h:65 * h + 65]
                        for i in ks:
                            bank_id, region = av_region(a, i - i0)
                            nc.tensor.matmul(
                                region,
                                at[:, off:off + 128],
                                lhs_v,
                                start=not primed[bank_id],
                                stop=(j == last_j[i]),
                                skip_group_check=True,
                            )
                            primed[bank_id] = True
                            off += 128

                for fi, plan in enumerate(plans):
                    # regroup plan entries [(j,i)...] -> [(j, ks)...]
                    fl = []
                    for j, i in plan:
                        if fl and fl[-1][0] == j:
                            fl[-1][1].append(i)
                        else:
                            fl.append((j, [i]))
                    for a in range(2):
                        sc_t, colw = emit_qk(a, fl)
                        emit_exp_av(a, fl, sc_t, colw)

                # ---- normalize + transpose this chunk (per head so head A's
                # transposes unlock while head B's normalize runs) ----
                rc = nrm_sb.tile([128, 8], F32, name="rc", tag="rc")
                nrm = nrm_sb.tile([128, 512], BF16, name="nrm", tag="nrm")
                tp = work_ps.tile([128, 512], BF16, name="tp", tag="w")
                for a in range(2):
                    nc.vector.reciprocal(rc[:, 4 * a:4 * a + 4], av[a][:, 64:260:65])
                    nc.vector.tensor_tensor(
                        nrm[:, a * 256:a * 256 + 256].rearrange("p (r c) -> p r c", c=64),
                        av[a][:, 0:260].rearrange("p (r c) -> p r c", c=65)[:, :, 0:64],
                        rc[:, 4 * a:4 * a + 4].unsqueeze(2).broadcast_to([128, 4, 64]),
                        mybir.AluOpType.mult,
                    )
                    for li in range(QC):
                        idx = a * QC + li
                        nc.tensor.transpose(
                            tp[a * 64:(a + 1) * 64, li * 128:(li + 1) * 128],
                            nrm[:, idx * 64:(idx + 1) * 64],
                            ident[:],
                        )
                nc.vector.tensor_copy(
                    outTbf[p][:, qc * 512:(qc + 1) * 512], tp[:])

        def final(ms, tail=False):
            # mid-attention groups: copies on DVE only (Act must stay free for
            # exp); the post-attention tail group splits copies across engines
            # and borrows the idle av-pool banks for more psum parallelism
            for mi, m in enumerate(ms):
                fsb = fin_sb.tile([128, 1024], BF16, name="fsb", tag="fsb")
                for n in range(2):
                    if tail and (2 * mi + n) % 2 == 1:
                        ps = av_ps.tile([128, 512], F32, name="f_av",
                                        tag=f"av{(2 * mi + n) // 2 % 2}")
                    else:
                        ps = work_ps.tile([128, 512], F32, name="f_ps", tag="w")
                    for p in range(2):
                        nc.tensor.matmul(
                            ps[:],
                            outTbf[p][:, m * 128:(m + 1) * 128],
                            wo[:, p * 1024 + n * 512: p * 1024 + (n + 1) * 512],
                            start=(p == 0), stop=(p == 1),
                        )
                    if tail and n == 1:
                        nc.scalar.copy(fsb[:, n * 512:(n + 1) * 512], ps[:])
                    else:
                        nc.vector.tensor_copy(fsb[:, n * 512:(n + 1) * 512], ps[:])
                nc.sync.dma_start(
                    outp_ap[m * 128:(m + 1) * 128, :], fsb[:])

        # emission order = scheduler priority. Interleave p0/p1 chunks and emit
        # each final m-group right after the (p1,qc) that completes its outT
        # columns, so final matmuls act as PE filler while later chunks stall
        # on the activation engine (exp).
        # Emission order = scheduler priority: attention QK outranks the bulk
        # projections (v, qk p1) so the Act engine is fed scores ASAP; the
        # lower-priority projections + final groups then fill PE stalls.
        # emission order = scheduler priority AND psum slot FIFO order; see
        # ORDER spec tokens: qk0/qk1, vA (m0-7), vB (m8-15), aPQ, F0-F2, F3t
        for tok in _order():
            if tok.startswith("#"):
                continue
            if tok == "qk0":
                proj_qk(0)
            elif tok == "qk1":
                proj_qk(1)
            elif tok == "v":
                proj_v()
            elif tok == "vA":
                proj_v(range(0, 8))
            elif tok == "vB":
                proj_v(range(8, 16))
            elif tok.startswith("a"):
                attn_chunk(int(tok[1]), int(tok[2]))
            elif tok.startswith("F"):
                g = int(tok[1])
                final([4 * g + k for k in range(4)], tail=tok.endswith("t"))
            else:
                raise ValueError(tok)


def _get_nc(kept):
    key = (kept, ORDER)
    if key in _nc_cache:
        return _nc_cache[key]
    nc = bacc.Bacc("TRN2", target_bir_lowering=False, debug=False, num_devices=NCORES)
    xhi_ap = nc.dram_tensor("xhi", [128, 16384], F8, kind="ExternalInput").ap()
    xlo_ap = nc.dram_tensor("xlo", [128, 16384], F8, kind="ExternalInput").ap()
    wq_ap = nc.dram_tensor("wqp", [128, 6144], F8, kind="ExternalInput").ap()
    wk_ap = nc.dram_tensor("wkp", [128, 6144], F8, kind="ExternalInput").ap()
    wv_ap = nc.dram_tensor("wvp", [128, 6144], F8, kind="ExternalInput").ap()
    wo_ap = nc.dram_tensor("wop", [128, 2048], BF16, kind="ExternalInput").ap()
    id_ap = nc.dram_tensor("ident", [128, 128], BF16, kind="ExternalInput").ap()
    outp_ap = nc.dram_tensor("outp", [S, D], BF16, kind="ExternalOutput").ap()
    with tile.TileContext(nc) as tc:
        _emit(tc, (xhi_ap, xlo_ap, wq_ap, wk_ap, wv_ap, wo_ap, id_ap, outp_ap),
              kept)
    nc.compile()
    _nc_cache[key] = nc
    return nc


def _pack_x_layout(t):
    # x[b].T f32 [1024,2048] -> [part, sc, kd, s-within] -> [128, 16384]
    t = t.reshape(KD, 128, 4, 512).transpose(1, 2, 0, 3)  # [128, 4, 8, 512]
    return np.ascontiguousarray(t.reshape(128, 16384))


def _pack_x_hilo(xb):
    a = np.ascontiguousarray(xb.T).astype(np.float32)     # [1024, 2048]
    hi = a.astype(f8)
    lo = ((a - hi.astype(np.float32)) * 16.0).astype(f8)
    return (_pack_x_layout(hi.astype(np.float32)).astype(f8),
            _pack_x_layout(lo.astype(np.float32)).astype(f8))


def _pack_w_layout(t):
    # [1024, 256] f32 -> [128, kd*256] f32
    t = t.reshape(KD, 128, 256).transpose(1, 0, 2)
    return np.ascontiguousarray(t.reshape(128, 2048))


def _w_terms(wslT):
    # -> (whi, wlo/16, whi/16) f32 arrays [1024, 256], fp8-quantized values
    a = (wslT * WS).astype(np.float32)
    whi = a.astype(f8)
    wlo16 = ((a - whi.astype(np.float32)) * 16.0).astype(f8)
    p2 = (wlo16.astype(np.float32) / 16.0).astype(f8)
    p3 = (whi.astype(np.float32) / 16.0).astype(f8)
    return (whi.astype(np.float32), p2.astype(np.float32), p3.astype(np.float32))


def _pack_w_comp(wslT):
    # W[sl,:].T [1024, 256] -> [whi | wlo/16 | whi/16] fp8 [128, 6144]
    out = np.concatenate([_pack_w_layout(t) for t in _w_terms(wslT)], axis=1)
    return np.ascontiguousarray(out).astype(f8)


def _pack_w_comp_qk(wslT):
    # W[sl,:].T [1024, 256] -> [p0: whi|wlo16|whi16 | p1: ...] fp8 [128, 6144]
    # per (p, term): [1024, 128] -> [128, kd*128]
    terms = _w_terms(wslT)
    packs = []
    for p in range(2):
        for t in terms:
            tp = t[:, p * 128:(p + 1) * 128].reshape(KD, 128, 128)
            packs.append(np.ascontiguousarray(
                tp.transpose(1, 0, 2).reshape(128, 1024)))
    return np.ascontiguousarray(np.concatenate(packs, axis=1)).astype(f8)


def _pack_wo(woT):
    # Wo[:,sl].T [256, 1024] -> [128, p*1024 + outcol]
    t = woT.reshape(2, 128, 1024).transpose(1, 0, 2)
    return np.ascontiguousarray(t.reshape(128, 2048)).astype(bf16)


def kernel(x, Wq, Wk, Wv, Wo, bo, block_mask):
    x = np.asarray(x, dtype=np.float32)
    Wq = np.asarray(Wq, dtype=np.float32)
    Wk = np.asarray(Wk, dtype=np.float32)
    Wv = np.asarray(Wv, dtype=np.float32)
    Wo = np.asarray(Wo, dtype=np.float32)
    bo = np.asarray(bo, dtype=np.float32)
    mask = np.asarray(block_mask).astype(bool)

    kept = tuple(tuple(int(j) for j in np.nonzero(mask[i])[0]) for i in range(NB))
    assert all(len(js) > 0 for js in kept), "a query block row has no kept blocks"

    t0 = time.monotonic()
    nc = _get_nc(kept)
    t_compile = time.monotonic() - t0

    ident = np.eye(128).astype(bf16)
    xp_b = [_pack_x_hilo(x[b]) for b in range(B)]
    in_maps = []
    for c in range(NCORES):
        b = c // (NCORES // B)
        hs = c % (NCORES // B)
        sl = slice(hs * E, (hs + 1) * E)
        in_maps.append({
            "xhi": xp_b[b][0],
            "xlo": xp_b[b][1],
            "wqp": _pack_w_comp_qk(np.ascontiguousarray(Wq[sl, :].T).astype(np.float32)),
            "wkp": _pack_w_comp_qk(np.ascontiguousarray(Wk[sl, :].T).astype(np.float32)),
            "wvp": _pack_w_comp(np.ascontiguousarray(Wv[sl, :].T).astype(np.float32)),
            "wop": _pack_wo(np.ascontiguousarray(
                (Wo[:, sl] / WS).T).astype(np.float32)),
            "ident": ident,
        })

    t0 = time.monotonic()
    res = run_bass_kernel_spmd(nc, in_maps, list(range(NCORES)))
    t_run = time.monotonic() - t0

    out = np.zeros((B, S, D), np.float32)
    for c in range(NCORES):
        out[c // (NCORES // B)] += res.results[c]["outp"].astype(np.float32)
    out += bo[None, None, :]

    last_run_info.update(compile_s=t_compile, run_s=t_run, nc=nc)
    return out

